# revision 14
# baseline (speedup 1.0000x reference)
"""Trainium2 Bass kernel for the Capsule routing layer (nn_Capsule_49658411876931).

Math (see reference):
    u_hat[b,j,i,d] = sum_k W[j,i,d,k] * x[b,i,k]
    b0 = 0
    for r in 0..2:
        c = softmax(b, axis=j)
        s[b,j,d] = sum_i c[b,j,i] u_hat[b,j,i,d]
        v = squash(s)  (over d)
        if r < 2: b += sum_d u_hat[b,j,i,d] v[b,j,d]
    return v  [B, J, D]

Sharding: input-capsule axis I=2048 split over 8 cores (I_LOC=256). W is
I-sharded, fp16 (2.1 MB/core). Only cross-core traffic: AllReduce of the
partial s [B, J*D] (fp16, 32 KB) per routing iteration, plus one warmup
AllReduce overlapped with phase 1 to absorb the collective cold-start.

Layouts (P = SBUF partition index), i_local = g*16 + r*4 + c:
  W       : [P = 32r + 8c + k, (g, d, j)]   full 128 partitions -> fast DMA
  x~      : [P = 32r + 8c' + k, (g, c, b)]  = x if c'==c else 0 (block-diag
            stationary; zero rows of the stationary kill the foreign-c
            rows of the shared [32, 512] moving W strip)
  x_dense : [P = 32r + 8c + k, (g, b)]      all-c stationary for the s0 sum
  u_hat C : [P = 32c + b, (g, r, d, j)] fp16
  logits  : [P = 32c + b, (g, r, j)]    fp16

Engine budget per routing iteration (DVE is the roofline engine):
  - weighted sum over i: DVE computes pi = c*C per 4-group chunk at 2x
    fp16; the (g,r,c-strip) reduction runs on the PE as 64 accumulating
    selector matmuls into one PSUM bank (no DVE reduce). Iteration 0
    (uniform c) needs no multiply at all: its s equals sum_i u_hat / J,
    accumulated during phase 1 by 64 extra PE matmuls (x_dense @ W).
  - agreement sum over d: DVE pairwise-add tree (4 levels) at 2x fp16.
  - GpSimd is deliberately unused for elementwise work: concurrent
    DVE+Pool tensor ops were measured to contend ~11x.
"""

import numpy as np

import concourse.bass as bass
import concourse.tile as tile
from concourse import bacc, mybir
from concourse.bass_utils import run_bass_kernel_spmd

F32 = mybir.dt.float32
F16 = mybir.dt.float16  # fp16: 11-bit mantissa, ample range here
U32 = mybir.dt.uint32
Alu = mybir.AluOpType
Act = mybir.ActivationFunctionType

B, I, K = 32, 2048, 8
J, D = 32, 16
JD = J * D                     # 512
NCORES = 8
I_LOC = I // NCORES            # 256
NG = I_LOC // 16               # 16 groups of 16 input capsules per core
NBLK = 4                       # W DMA blocks (4 groups each)
GPB = NG // NBLK
NCH = 4                        # routing chunk tile size (groups)
GPC = NG // NCH
CHUNKS = (4, 4, 4, 3, 1)       # tail-light chunk plan (sums to NG)
ROUTINGS = 3
EPS = 1e-7

_CACHE = {}


def _build():
    nc = bacc.Bacc("TRN2", target_bir_lowering=False, debug=False, num_devices=NCORES)

    wt_in = nc.dram_tensor("wt", [NBLK, 128, GPB, JD], F16, kind="ExternalInput")
    xs_in = nc.dram_tensor("xs", [128, NG, 4, B], F16, kind="ExternalInput")
    xd_in = nc.dram_tensor("xd", [128, NG, B], F16, kind="ExternalInput")
    v_out = nc.dram_tensor("v", [B, J, D], F32, kind="ExternalOutput")

    # f32 constant block: selT (v broadcast), rsqrt uint32 constants.
    cst32_np = np.zeros((128, 192), np.float32)
    sel_np = np.zeros((128, B), np.float32)
    sel_np[np.arange(128), np.arange(128) % B] = 1.0
    cst32_np[0:B, 0:128] = sel_np.T                   # selT[b, p]
    cst32_np[0:B, 128:160] = np.full((B, J), 0x5F3759DF, np.uint32).view(np.float32)
    cst32_np[0:B, 160:192] = np.ones((B, J), np.uint32).view(np.float32)
    cst32_dram = nc.inline_tensor(cst32_np, "cst32")
    # fp16 constant block: sel (strip collapse)
    cst16_np = sel_np.astype(np.float16)
    cst16_dram = nc.inline_tensor(cst16_np, "cst16")

    with tile.TileContext(nc) as tc:
        with (
            tc.tile_pool(name="persist", bufs=1) as pp,
            tc.tile_pool(name="small", bufs=1) as sp,
            tc.tile_pool(name="dram", bufs=1, space="DRAM") as dp,
        ):
            # ---- persistent SBUF tensors ----
            xs = pp.tile([128, NG, 4, B], F16)          # block-diag x~
            xd = pp.tile([128, NG, B], F16)             # dense x (s0 stationary)
            C = pp.tile([128, NG, 4, D, J], F16)        # u_hat, (d, j) free layout
            bl = pp.tile([128, NG, 4, J], F16)          # routing logits
            c_sb = pp.tile([128, NG, 4, J], F16)        # softmax coefficients
            p_t = pp.tile([128, NG, 4, J], F16)         # exp(b)
            cst32 = pp.tile([128, 192], F32)
            cst16 = pp.tile([128, B], F16)
            v_rep = pp.tile([128, D, J], F16)           # v replicated over c-strips

            selT = cst32[0:B, 0:128]
            magic = cst32[0:B, 128:160].bitcast(U32)
            oneu = cst32[0:B, 160:192].bitcast(U32)
            sel = cst16[:, 0:B]

            nc.sync.dma_start(cst32[:], cst32_dram[:])
            nc.sync.dma_start(cst16[:], cst16_dram[:])
            nc.sync.dma_start(xs[:], xs_in[:])
            nc.sync.dma_start(xd[:], xd_in[:])
            nc.vector.memset(bl[:], 0.0)
            # Funnel all initial-load waits through one barrier so the first
            # matmuls don't exceed the per-instruction sync-wait budget.
            tc.strict_bb_all_engine_barrier()

            # Warm the collective path during phase 1 so the first real
            # AllReduce doesn't pay the cold-start cost.
            cw_in = dp.tile([1, 4], F32, tag="cw_in")
            cw_out = dp.tile([1, 4], F32, tag="cw_out", addr_space="Shared")
            warm = sp.tile([1, 4], F32, tag="warm")
            nc.vector.memset(warm[:], 0.0)
            nc.gpsimd.dma_start(cw_in[:], warm[:])
            nc.gpsimd.collective_compute(
                "AllReduce",
                Alu.add,
                replica_groups=[list(range(NCORES))],
                ins=[cw_in.opt()],
                outs=[cw_out.opt()],
            )

            # ---- phase 1: u_hat + s0 = sum_i u_hat (PE-accumulated) ----
            with (
                tc.tile_pool(name="wpool", bufs=1) as wp,
                tc.tile_pool(name="psum1", bufs=3, space="PSUM") as ps1,
                tc.tile_pool(name="psum_s", bufs=1, space="PSUM") as pss,
            ):
                s0_ps = pss.tile([B, JD], F32, tag="s0")
                wts = []
                for blk in range(NBLK):
                    w_b = wp.tile([128, GPB, JD], F16, tag=f"wt{blk}")
                    nc.sync.dma_start(w_b[:], wt_in[blk])
                    wts.append(w_b)
                for g in range(NG):
                    blk, g4 = divmod(g, GPB)
                    w_b = wts[blk]
                    # u_hat matmuls in two r-halves (2 PSUM banks each)
                    for h in range(2):
                        ph = ps1.tile([128, 2, JD], F32, tag="ps")
                        for r2 in range(2):
                            r = h * 2 + r2
                            for c in range(4):
                                nc.tensor.matmul(
                                    ph[32 * c : 32 * c + 32, r2, :],
                                    xs[32 * r : 32 * r + 32, g, c, :],
                                    w_b[32 * r : 32 * r + 32, g4, :],
                                    tile_position=(32 * r, 32 * c),
                                )
                        # evacuate psum half (contiguous) to fp16 C
                        src = ph.rearrange("p r (d j) -> p r d j", j=J, d=D)
                        dst = C[:, g, 2 * h : 2 * h + 2]
                        if h == 0:
                            nc.scalar.copy(dst, src)
                        else:
                            nc.vector.tensor_copy(dst, src)
                # s0 = sum_i u_hat: one K=128 matmul per group (contraction
                # over all (r, c, k) rows at once), contiguous accumulation
                # group so it can't interleave with the u_hat groups above.
                for g in range(NG):
                    blk, g4 = divmod(g, GPB)
                    nc.tensor.matmul(
                        s0_ps[:],
                        xd[:, g, :],
                        wts[blk][:, g4, :],
                        start=(g == 0),
                        stop=(g == NG - 1),
                    )
                # scale s0 by 1/J while evacuating
                s_loc0 = sp.tile([B, JD], F32, tag="s_loc")
                nc.scalar.mul(s_loc0[:], s0_ps[:], 1.0 / J)

            # ---- routing ----
            with (
                tc.tile_pool(name="chpool", bufs=2) as chp,
                tc.tile_pool(name="psum2", bufs=2, space="PSUM") as ps2,
            ):
                for it in range(ROUTINGS):
                    if it > 0:
                        # ---- agreement: bl += sum_d C * v_rep, chunked ----
                        # tail-light chunk plan: softmax can start almost
                        # immediately after the last (1-group) chunk lands.
                        g0 = 0
                        for ng in CHUNKS:
                            pi2 = chp.tile([128, GPC, 4, D, J], F16, tag="pi2")
                            nc.vector.tensor_tensor(
                                pi2[:, 0:ng],
                                C[:, g0 : g0 + ng],
                                v_rep[:, None, None, :, :].broadcast_to(
                                    [128, ng, 4, D, J]
                                ),
                                op=Alu.mult,
                            )
                            t1 = chp.tile([128, GPC, 4, 8, J], F16, tag="t1")
                            nc.vector.tensor_tensor(
                                t1[:, 0:ng], pi2[:, 0:ng, :, 0:8],
                                pi2[:, 0:ng, :, 8:16], op=Alu.add,
                            )
                            t2 = chp.tile([128, GPC, 4, 4, J], F16, tag="t2")
                            nc.vector.tensor_tensor(
                                t2[:, 0:ng], t1[:, 0:ng, :, 0:4],
                                t1[:, 0:ng, :, 4:8], op=Alu.add,
                            )
                            t3 = chp.tile([128, GPC, 4, 2, J], F16, tag="t3")
                            nc.vector.tensor_tensor(
                                t3[:, 0:ng], t2[:, 0:ng, :, 0:2],
                                t2[:, 0:ng, :, 2:4], op=Alu.add,
                            )
                            a_c = chp.tile([128, GPC, 4, J], F16, tag="a_c")
                            nc.vector.tensor_tensor(
                                a_c[:, 0:ng], t3[:, 0:ng, :, 0], t3[:, 0:ng, :, 1],
                                op=Alu.add,
                            )
                            nc.vector.tensor_add(
                                bl[:, g0 : g0 + ng], bl[:, g0 : g0 + ng],
                                a_c[:, 0:ng],
                            )
                            # overlap exp (ACT) with the next chunk's DVE work
                            nc.scalar.activation(
                                p_t[:, g0 : g0 + ng], bl[:, g0 : g0 + ng], Act.Exp
                            )
                            g0 += ng

                        # ---- softmax over j (free axis) ----
                        S = sp.tile([128, NG, 4], F32, tag="S")
                        nc.vector.tensor_reduce(
                            S[:], p_t[:], axis=mybir.AxisListType.X, op=Alu.add
                        )
                        Sr = sp.tile([128, NG, 4], F32, tag="Sr")
                        nc.vector.reciprocal(Sr[:], S[:])
                        nc.vector.tensor_tensor(
                            c_sb[:],
                            p_t[:],
                            Sr[:, :, :, None].broadcast_to([128, NG, 4, J]),
                            op=Alu.mult,
                        )

                        # ---- s = sum_i c*u_hat : DVE mult + PE accumulate ----
                        s_ps = ps2.tile([B, JD], F32, tag="s_ps")
                        mm = 0
                        g0 = 0
                        for ng in CHUNKS:
                            pic = chp.tile([128, GPC, 4, D, J], F16, tag="pic")
                            nc.vector.tensor_tensor(
                                pic[:, 0:ng],
                                C[:, g0 : g0 + ng],
                                c_sb[:, g0 : g0 + ng, :, None, :].broadcast_to(
                                    [128, ng, 4, D, J]
                                ),
                                op=Alu.mult,
                            )
                            for g4 in range(ng):
                                for r in range(4):
                                    nc.tensor.matmul(
                                        s_ps[:],
                                        sel,
                                        pic[:, g4, r].rearrange("p d j -> p (d j)"),
                                        start=(mm == 0),
                                        stop=(mm == NG * 4 - 1),
                                    )
                                    mm += 1
                            g0 += ng
                        s_loc = sp.tile([B, JD], F32, tag="s_loc")
                        nc.scalar.copy(s_loc[:], s_ps[:])
                    else:
                        s_loc = s_loc0

                    # AllReduce partial s over the 8 cores (fp16 payload)
                    cc_in = dp.tile([B, JD], F32, tag="cc_in")
                    cc_out = dp.tile([B, JD], F32, tag="cc_out", addr_space="Shared")
                    s_glob = sp.tile([B, D, J], F32, tag="s_glob")
                    nc.sync.dma_start(cc_in[:], s_loc[:])
                    nc.gpsimd.collective_compute(
                        "AllReduce",
                        Alu.add,
                        replica_groups=[list(range(NCORES))],
                        ins=[cc_in.opt()],
                        outs=[cc_out.opt()],
                    )
                    nc.sync.dma_start(
                        s_glob.rearrange("b d j -> b (d j)"), cc_out[:]
                    )

                    # ---- squash on [B, D, J] (all cores redundantly) ----
                    sq = sp.tile([B, D, J], F32, tag="sq")
                    nc.vector.tensor_tensor(sq[:], s_glob[:], s_glob[:], op=Alu.mult)
                    n2 = sp.tile([B, J], F32, tag="n2")
                    nc.vector.tensor_reduce(
                        n2[:],
                        sq.rearrange("b d j -> b j d"),
                        axis=mybir.AxisListType.X,
                        op=Alu.add,
                    )
                    # fast inverse sqrt + 2 Newton steps (DVE only, no ACT
                    # tables); n2 >> EPS here so the seed shift uses n2 directly
                    xh = sp.tile([B, J], F32, tag="xh")
                    nc.vector.tensor_scalar(
                        xh[:], n2[:], EPS, 0.5, op0=Alu.add, op1=Alu.mult
                    )
                    rsq = sp.tile([B, J], F32, tag="rsq")
                    tmp = sp.tile([B, J], F32, tag="tmp")
                    nc.vector.tensor_tensor(
                        tmp.bitcast(U32), n2.bitcast(U32), oneu,
                        op=Alu.logical_shift_right,
                    )
                    nc.vector.tensor_tensor(
                        rsq.bitcast(U32), magic, tmp.bitcast(U32), op=Alu.subtract
                    )
                    for _ in range(2):
                        nc.vector.tensor_tensor(tmp[:], rsq[:], rsq[:], op=Alu.mult)
                        nc.vector.tensor_tensor(tmp[:], xh[:], tmp[:], op=Alu.mult)
                        nc.vector.tensor_scalar(
                            tmp[:], tmp[:], -1.0, 1.5, op0=Alu.mult, op1=Alu.add
                        )
                        nc.vector.tensor_tensor(rsq[:], rsq[:], tmp[:], op=Alu.mult)
                    # factor = n2 / (1 + n2) * rsq
                    fac = sp.tile([B, J], F32, tag="fac")
                    nc.vector.tensor_scalar_add(tmp[:], n2[:], 1.0)
                    nc.vector.reciprocal(fac[:], tmp[:])
                    nc.vector.tensor_tensor(fac[:], fac[:], n2[:], op=Alu.mult)
                    nc.vector.tensor_tensor(fac[:], fac[:], rsq[:], op=Alu.mult)
                    v_f = sp.tile([B, D, J], F32, tag="v_f")
                    nc.vector.tensor_tensor(
                        v_f[:],
                        s_glob[:],
                        fac[:, None, :].broadcast_to([B, D, J]),
                        op=Alu.mult,
                    )

                    if it < ROUTINGS - 1:
                        # replicate v over the 4 c-strips via PE
                        vr_ps = ps2.tile([128, D * J], F32, tag="vr_ps")
                        nc.tensor.matmul(
                            vr_ps[:], selT, v_f.rearrange("b d j -> b (d j)")
                        )
                        nc.scalar.copy(
                            v_rep.rearrange("p d j -> p (d j)"), vr_ps[:]
                        )
                    else:
                        # final output: reorder (d, j) -> (j, d) and store
                        v_jd = sp.tile([B, J, D], F32, tag="v_jd")
                        nc.vector.tensor_copy(
                            v_jd[:], v_f.rearrange("b d j -> b j d")
                        )
                        nc.sync.dma_start(v_out[:], v_jd[:])

    nc.compile()
    return nc


def _prep_inputs(x, W):
    """Per-core host-side sharding + layout prep (fp16)."""
    in_maps = []
    for m in range(NCORES):
        lo, hi = m * I_LOC, (m + 1) * I_LOC
        Wc = W[:, lo:hi]                       # [J, 256, D, K]
        Wc = Wc.reshape(J, NBLK, GPB, 4, 4, D, K)  # i = (blk*GPB+g4)*16+r*4+c
        # -> [blk, (r, c, k) = partition, g4, d, j]
        wt = np.ascontiguousarray(Wc.transpose(1, 3, 4, 6, 2, 5, 0)).reshape(
            NBLK, 128, GPB, JD
        )
        xc = x[:, lo:hi, :].reshape(B, NG, 4, 4, K)   # [b, g, r, c, k]
        xsrc = xc.transpose(2, 3, 4, 1, 0).astype(np.float16)  # [r, c, k, g, b]
        # xt[32r+8c'+k, g, c, b] = x[b, g, r, c, k] if c'==c else 0
        xt = np.zeros((4, 4, K, NG, 4, B), np.float16)  # [r, c', k, g, c, b]
        for c in range(4):
            xt[:, c, :, :, c, :] = xsrc[:, c]
        xt = xt.reshape(128, NG, 4, B)
        # dense variant: xd[32r+8c+k, g, b] = x[b, g, r, c, k]
        xd = np.ascontiguousarray(xsrc).reshape(128, NG, B)
        in_maps.append(
            {"wt": wt.astype(np.float16), "xs": xt, "xd": xd}
        )
    return in_maps


def run(inputs, trace=False):
    if "nc" not in _CACHE:
        _CACHE["nc"] = _build()
    nc = _CACHE["nc"]
    in_maps = _prep_inputs(inputs["x"], inputs["W"])
    bkr = run_bass_kernel_spmd(
        nc, in_maps, core_ids=list(range(NCORES)), trace=trace
    )
    out = bkr.results[0]["v"].astype(np.float32)
    return out, bkr


def kernel(x, W):
    out, _ = run({"x": np.asarray(x), "W": np.asarray(W)})
    return out
